# revision 1
# baseline (speedup 1.0000x reference)
"""MoEConv Trainium2 kernel (8 NeuronCores, SPMD).

Strategy (dst-sharded, fully dense device program):
- Host: shard destination nodes across 8 cores (degree-balanced), group each
  core's edges by dst node into fixed-window padded "slots" (window uniform
  per 256-slot block, groups never straddle 128-slot halves). Ship per-slot
  x[src] (transposed, bf16) and pos[src]/pos[dst] (f32).
- Device per core, all dense ops:
  * gating: logits = (pos_s - pos_d) @ gate_W + b, top-2 masked softmax ->
    per-slot weight row Kw[slot, 8] (zeros except top-2).
  * Z = x_j @ [W_0|...|W_7]  (one PE matmul per 128-slot tile -> PSUM [128,512])
  * msg = sum_k Kw[:,k] * Z[:,k*64:(k+1)*64]   (DVE mul/add chain)
  * PE pair-transpose msg -> PSUM [128,128]; windowed reduce_max -> ACC cols
  * MLP on ACC (transposed), skip add; host reassembles/unpermutes.
No indirect DMA, no collectives.
"""

import math
from contextlib import ExitStack

import numpy as np

import concourse.bacc as bacc
import concourse.bass as bass
import concourse.tile as tile
from concourse import mybir
from concourse.bass_utils import run_bass_kernel_spmd
from concourse.masks import make_identity

P = 128
N_CORES = 8
IN_C = 64
OUT_C = 64
NK = 8
DIM = 2
BIG = 1.0e30
BF16 = mybir.dt.bfloat16
F32 = mybir.dt.float32


# ---------------------------------------------------------------- host layout
class Layout:
    pass


def build_layout(dst, src, N):
    """Compute the shared (across cores) block schedule and per-core slot
    arrays. Returns Layout with per-core: slot_src, slot_dst (int32 [SL]),
    and shared: block windows w[], caps g[], col offsets, plus output-mapping
    (core, node, half, col) arrays."""
    deg = np.bincount(dst, minlength=N)
    assert deg.max() <= P, f"max degree {deg.max()} > 128 unsupported"
    order = np.argsort(-deg, kind="stable")  # global degree-descending
    core_of_node = np.empty(N, dtype=np.int64)
    core_of_node[order] = np.arange(N) % N_CORES

    # per-core node lists (degree-descending)
    nodes_c = [order[core_of_node[order] == c] for c in range(N_CORES)]

    # per-core edge lists grouped by node in list order
    edge_core = core_of_node[dst]
    # rank of node within its core list
    rank_in_core = np.empty(N, dtype=np.int64)
    for c in range(N_CORES):
        rank_in_core[nodes_c[c]] = np.arange(len(nodes_c[c]))

    # group edges: sort each core's edges by rank_in_core[dst]
    edges_c = []
    for c in range(N_CORES):
        idx = np.nonzero(edge_core == c)[0]
        o = np.argsort(rank_in_core[dst[idx]], kind="stable")
        edges_c.append(idx[o])

    # ---- shared block schedule (iterate to fixpoint) ----
    degs_c = [deg[nodes_c[c]] for c in range(N_CORES)]
    w = []  # shared per-block window

    def pack(core_degs, wseq):
        """Greedily pack nodes into blocks given (possibly partial) wseq.
        Returns list of per-block node counts and the per-block max degree."""
        counts, maxdeg = [], []
        i, nblk = 0, 0
        n = len(core_degs)
        while i < n:
            if nblk < len(wseq):
                wb = max(wseq[nblk], int(core_degs[i]))
            else:
                wb = int(core_degs[i])
            cap = 2 * (P // wb)
            take = min(cap, n - i)
            counts.append(take)
            maxdeg.append(int(core_degs[i]))  # degree-desc => first is max
            i += take
            nblk += 1
        return counts, maxdeg

    for _ in range(20):
        allmax = []
        for c in range(N_CORES):
            _, md = pack(degs_c[c], w)
            allmax.append(md)
        B = max(len(m) for m in allmax)
        neww = []
        for j in range(B):
            cand = [m[j] for m in allmax if j < len(m)]
            wj = max(cand + ([w[j]] if j < len(w) else []))
            neww.append(wj)
        if neww == w:
            break
        w = neww
    w = np.array(w, dtype=np.int64)
    B = len(w)
    g = P // w  # groups (nodes) per 128-half
    colb = np.concatenate([[0], np.cumsum(g)])  # ACC col offset per block
    C = int(colb[-1])

    # ---- per-core slot arrays ----
    lay = Layout()
    lay.B, lay.w, lay.g, lay.colb, lay.C = B, w, g, colb, C
    lay.SL = B * 256
    lay.slot_src, lay.slot_dst = [], []
    lay.out_node, lay.out_half, lay.out_col, lay.out_core = [], [], [], []
    for c in range(N_CORES):
        nodes = nodes_c[c]
        ecs = edges_c[c]
        esrc = src[ecs]
        edst = dst[ecs]
        # edge start offset per node (grouped!)
        dcs = deg[nodes]
        starts = np.concatenate([[0], np.cumsum(dcs)])
        s_src = np.zeros(lay.SL, dtype=np.int32)
        s_dst = np.zeros(lay.SL, dtype=np.int32)
        ni = 0  # node cursor
        for b in range(B):
            wb, gb = int(w[b]), int(g[b])
            base = b * 256
            for h in range(2):
                hbase = base + h * P
                for m in range(gb):
                    lo = hbase + m * wb
                    if ni < len(nodes):
                        node = nodes[ni]
                        st, en = starts[ni], starts[ni + 1]
                        d = en - st
                        take = min(d, wb)
                        s_src[lo:lo + take] = esrc[st:st + take]
                        s_dst[lo:lo + take] = edst[st:st + take]
                        if take < wb:  # pad: duplicate first edge
                            s_src[lo + take:lo + wb] = esrc[st]
                            s_dst[lo + take:lo + wb] = edst[st]
                        lay.out_node.append(node)
                        lay.out_half.append(h)
                        lay.out_col.append(colb[b] + m)
                        lay.out_core.append(c)
                        ni += 1
                    else:  # dummy group: duplicate previous slot content
                        s_src[lo:lo + wb] = s_src[lo - 1] if lo > 0 else 0
                        s_dst[lo:lo + wb] = s_dst[lo - 1] if lo > 0 else 0
                # tail pad of the half (128 - gb*wb slots)
                lo = hbase + gb * wb
                if lo < hbase + P:
                    s_src[lo:hbase + P] = s_src[lo - 1] if lo > 0 else 0
                    s_dst[lo:hbase + P] = s_dst[lo - 1] if lo > 0 else 0
        assert ni == len(nodes), (ni, len(nodes))
        lay.slot_src.append(s_src)
        lay.slot_dst.append(s_dst)
    lay.out_node = np.array(lay.out_node)
    lay.out_half = np.array(lay.out_half)
    lay.out_col = np.array(lay.out_col)
    lay.out_core = np.array(lay.out_core)
    return lay


# ------------------------------------------------------------- device program
def build_program(lay, repeat=1):
    T = lay.SL // P          # 128-slot tiles
    B = lay.B                # 256-slot blocks (2 tiles)
    GST = 32                 # tiles per gating supertile
    n_gs = math.ceil(T / GST)
    T_pad = n_gs * GST
    XC = 32                  # tiles per x-chunk load
    C = lay.C
    CP = math.ceil(C / 512) * 512  # padded ACC cols for MLP chunks

    nc = bacc.Bacc("TRN2", target_bir_lowering=False, debug=False,
                   num_devices=N_CORES)
    # inputs
    xjT = nc.dram_tensor("xjT", [IN_C, lay.SL], BF16, kind="ExternalInput")
    pos4 = nc.dram_tensor("pos4", [P, T_pad, 4], F32, kind="ExternalInput")
    wcat = nc.dram_tensor("wcat", [IN_C, NK * OUT_C], BF16, kind="ExternalInput")
    gw0t = nc.dram_tensor("gw0t", [P, GST, NK], F32, kind="ExternalInput")
    gw1t = nc.dram_tensor("gw1t", [P, GST, NK], F32, kind="ExternalInput")
    bt = nc.dram_tensor("bt", [P, GST, NK], F32, kind="ExternalInput")
    w1 = nc.dram_tensor("w1", [OUT_C, 2 * OUT_C], F32, kind="ExternalInput")
    w2 = nc.dram_tensor("w2", [2 * OUT_C, OUT_C], F32, kind="ExternalInput")
    outd = nc.dram_tensor("out", [P, C], F32, kind="ExternalOutput")

    with tile.TileContext(nc) as tc, ExitStack() as ctx:
        cpool = ctx.enter_context(tc.tile_pool(name="consts", bufs=1))
        xpool = ctx.enter_context(tc.tile_pool(name="xc", bufs=2))
        ppool = ctx.enter_context(tc.tile_pool(name="pos", bufs=2))
        gpool = ctx.enter_context(tc.tile_pool(name="gate", bufs=2))
        kwpool = ctx.enter_context(tc.tile_pool(name="kw", bufs=3))
        msgp = ctx.enter_context(tc.tile_pool(name="msg", bufs=3))
        zp = ctx.enter_context(tc.tile_pool(name="z", bufs=2, space="PSUM"))
        tp = ctx.enter_context(tc.tile_pool(name="tp", bufs=4, space="PSUM"))
        accp = ctx.enter_context(tc.tile_pool(name="acc", bufs=1))
        mlpp = ctx.enter_context(tc.tile_pool(name="mlp", bufs=2))

        wcat_s = cpool.tile([IN_C, NK * OUT_C], BF16)
        nc.sync.dma_start(wcat_s[:], wcat[:])
        gw0_s = cpool.tile([P, GST, NK], F32)
        nc.sync.dma_start(gw0_s[:], gw0t[:])
        gw1_s = cpool.tile([P, GST, NK], F32)
        nc.sync.dma_start(gw1_s[:], gw1t[:])
        bt_s = cpool.tile([P, GST, NK], F32)
        nc.sync.dma_start(bt_s[:], bt[:])
        w1_s = cpool.tile([OUT_C, 2 * OUT_C], F32)
        nc.sync.dma_start(w1_s[:], w1[:])
        w2_s = cpool.tile([2 * OUT_C, OUT_C], F32)
        nc.sync.dma_start(w2_s[:], w2[:])
        ident = cpool.tile([P, P], BF16)
        make_identity(nc, ident[:])

        acc = accp.tile([P, C], F32)

        # ---------- gating: Kw supertiles ----------
        kws = []
        for gsi in range(n_gs):
            t0 = gsi * GST
            pos_t = ppool.tile([P, GST, 4], F32)
            nc.sync.dma_start(pos_t[:], pos4[:, t0:t0 + GST])
            dp = gpool.tile([P, GST, DIM], F32)
            nc.vector.tensor_tensor(out=dp[:], in0=pos_t[:, :, 0:2],
                                    in1=pos_t[:, :, 2:4],
                                    op=mybir.AluOpType.subtract)
            lg = gpool.tile([P, GST, NK], F32)
            nc.vector.tensor_tensor(
                out=lg[:], in0=dp[:, :, 0:1].to_broadcast([P, GST, NK]),
                in1=gw0_s[:], op=mybir.AluOpType.mult)
            t2t = gpool.tile([P, GST, NK], F32)
            nc.vector.tensor_tensor(
                out=t2t[:], in0=dp[:, :, 1:2].to_broadcast([P, GST, NK]),
                in1=gw1_s[:], op=mybir.AluOpType.mult)
            nc.vector.tensor_tensor(out=lg[:], in0=lg[:], in1=t2t[:],
                                    op=mybir.AluOpType.add)
            nc.vector.tensor_tensor(out=lg[:], in0=lg[:], in1=bt_s[:],
                                    op=mybir.AluOpType.add)
            v1 = gpool.tile([P, GST], F32)
            nc.vector.tensor_reduce(out=v1[:], in_=lg[:],
                                    axis=mybir.AxisListType.X,
                                    op=mybir.AluOpType.max)
            m1 = gpool.tile([P, GST, NK], F32)
            nc.vector.tensor_tensor(
                out=m1[:], in0=lg[:],
                in1=v1[:, :, None].to_broadcast([P, GST, NK]),
                op=mybir.AluOpType.is_equal)
            lg2 = gpool.tile([P, GST, NK], F32)
            nc.vector.scalar_tensor_tensor(
                out=lg2[:], in0=m1[:], scalar=-BIG, in1=lg[:],
                op0=mybir.AluOpType.mult, op1=mybir.AluOpType.add)
            v2 = gpool.tile([P, GST], F32)
            nc.vector.tensor_reduce(out=v2[:], in_=lg2[:],
                                    axis=mybir.AxisListType.X,
                                    op=mybir.AluOpType.max)
            m2 = gpool.tile([P, GST, NK], F32)
            nc.vector.tensor_tensor(
                out=m2[:], in0=lg2[:],
                in1=v2[:, :, None].to_broadcast([P, GST, NK]),
                op=mybir.AluOpType.is_equal)
            d = gpool.tile([P, GST], F32)
            nc.vector.tensor_tensor(out=d[:], in0=v2[:], in1=v1[:],
                                    op=mybir.AluOpType.subtract)
            e = gpool.tile([P, GST], F32)
            nc.scalar.activation(e[:], d[:], mybir.ActivationFunctionType.Exp)
            s1 = gpool.tile([P, GST], F32)
            nc.vector.tensor_scalar(out=s1[:], in0=e[:], scalar1=1.0,
                                    scalar2=None, op0=mybir.AluOpType.add)
            r = gpool.tile([P, GST], F32)
            nc.vector.reciprocal(r[:], s1[:])
            w2v = gpool.tile([P, GST], F32)
            nc.vector.tensor_tensor(out=w2v[:], in0=e[:], in1=r[:],
                                    op=mybir.AluOpType.mult)
            kw = kwpool.tile([P, GST, NK], F32)
            nc.vector.tensor_tensor(
                out=kw[:], in0=m1[:],
                in1=r[:, :, None].to_broadcast([P, GST, NK]),
                op=mybir.AluOpType.mult)
            kw2 = gpool.tile([P, GST, NK], F32)
            nc.vector.tensor_tensor(
                out=kw2[:], in0=m2[:],
                in1=w2v[:, :, None].to_broadcast([P, GST, NK]),
                op=mybir.AluOpType.mult)
            nc.vector.tensor_tensor(out=kw[:], in0=kw[:], in1=kw2[:],
                                    op=mybir.AluOpType.add)
            kw16 = kwpool.tile([P, GST, NK], BF16, tag="kw16")
            nc.vector.tensor_copy(out=kw16[:], in_=kw[:])
            kws.append(kw16)

        # ---------- main loop over blocks ----------
        xc = None
        for rep in range(repeat):
         for b in range(B):
            t0 = 2 * b
            if t0 % XC == 0:
                xc = xpool.tile([IN_C, XC * P], BF16)
                lo = t0 * P
                hi = min(lo + XC * P, lay.SL)
                nc.sync.dma_start(xc[:, :hi - lo], xjT[:, lo:hi])
            z = zp.tile([P, 2, 512], F32, space="PSUM")
            for i in range(2):
                off = ((t0 + i) * P) % (XC * P)
                nc.tensor.matmul(out=z[:, i], lhsT=xc[:, off:off + P],
                                 rhs=wcat_s[:], start=True, stop=True)
            kw16 = kws[t0 // GST]
            gg = t0 % GST  # first tile's group index within supertile
            zsb = msgp.tile([P, 2, NK * OUT_C], BF16, tag="zsb")
            nc.scalar.copy(out=zsb[:], in_=z[:])
            prod = msgp.tile([P, NK, 2, OUT_C], BF16, tag="prod")
            kwb = kw16[:, gg:gg + 2, :, None].to_broadcast([P, 2, NK, OUT_C])
            nc.vector.tensor_tensor(
                out=prod[:].rearrange("p k a c -> p a k c"),
                in0=zsb[:].rearrange("p a (k c) -> p a k c", k=NK),
                in1=kwb, op=mybir.AluOpType.mult)
            # sum over k via accumulating transpose-matmuls -> msg^T pair
            tps = tp.tile([P, P], F32, space="PSUM")
            for k in range(NK):
                nc.tensor.matmul(
                    out=tps[:], lhsT=prod[:, k].rearrange("p a c -> p (a c)"),
                    rhs=ident[:], start=(k == 0), stop=(k == NK - 1))
            wb, gb = int(lay.w[b]), int(lay.g[b])
            cb = int(lay.colb[b])
            nc.vector.tensor_reduce(
                out=acc[:, cb:cb + gb],
                in_=tps[:, :gb * wb].rearrange("p (g w) -> p g w", g=gb),
                axis=mybir.AxisListType.X, op=mybir.AluOpType.max)

        # ---------- MLP on ACC ----------
        hout = mlpp.tile([P, CP], F32, tag="hout")
        acc_lo = mlpp.tile([OUT_C, C], F32, tag="acclo")
        nc.sync.dma_start(acc_lo[:], acc[OUT_C:2 * OUT_C, :])
        for h in range(2):
            oh = acc[0:OUT_C, :] if h == 0 else acc_lo[:]  # [64, C]
            for j in range(0, C, 512):
                je = min(j + 512, C)
                u = zp.tile([P, 2, 512], F32, space="PSUM", tag="z")
                nc.tensor.matmul(out=u[:, 0, :je - j], lhsT=w1_s[:],
                                 rhs=oh[:, j:je], start=True, stop=True)
                rl = mlpp.tile([P, 512], F32, tag="relu")
                nc.scalar.activation(rl[:, :je - j], u[:, 0, :je - j],
                                     mybir.ActivationFunctionType.Relu)
                v = tp.tile([OUT_C, 512], F32, space="PSUM", tag="tps")
                nc.tensor.matmul(out=v[:, :je - j], lhsT=w2_s[:],
                                 rhs=rl[:, :je - j], start=True, stop=True)
                nc.vector.tensor_tensor(
                    out=hout[h * OUT_C:(h + 1) * OUT_C, j:je],
                    in0=v[:, :je - j], in1=oh[:, j:je],
                    op=mybir.AluOpType.add)
        nc.sync.dma_start(outd[:, :], hout[:, :C])

    nc.compile()
    return nc


# ------------------------------------------------------------------ top level
def _build_inputs(lay, x, pos, expert_weights, gate_W, gate_b, W1, W2):
    T = lay.SL // P
    GST = 32
    T_pad = math.ceil(T / GST) * GST
    wcat = np.ascontiguousarray(
        expert_weights.transpose(1, 0, 2).reshape(IN_C, NK * OUT_C)
    ).astype(np.float32)
    gw0t = np.tile(gate_W[0].astype(np.float32), (P, GST, 1))
    gw1t = np.tile(gate_W[1].astype(np.float32), (P, GST, 1))
    bt = np.tile(gate_b.astype(np.float32), (P, GST, 1))
    import ml_dtypes
    in_maps = []
    for c in range(N_CORES):
        ssrc = lay.slot_src[c]
        sdst = lay.slot_dst[c]
        xjT = np.ascontiguousarray(x[ssrc].T).astype(ml_dtypes.bfloat16)
        # pos4[p, t] = [pos_src, pos_dst] for slot t*128+p
        pos4 = np.concatenate([pos[ssrc], pos[sdst]], axis=1)  # [SL, 4]
        pos4 = pos4.reshape(T, P, 4).transpose(1, 0, 2)
        if T_pad > T:
            pos4 = np.concatenate(
                [pos4, np.zeros((P, T_pad - T, 4), pos4.dtype)], axis=1)
        pos4 = np.ascontiguousarray(pos4).astype(np.float32)
        in_maps.append({
            "xjT": xjT,
            "pos4": pos4,
            "wcat": wcat.astype(ml_dtypes.bfloat16),
            "gw0t": gw0t, "gw1t": gw1t, "bt": bt,
            "w1": W1.astype(np.float32), "w2": W2.astype(np.float32),
        })
    return in_maps


def kernel(x, pos, edge_index, expert_weights, gate_W, gate_b, W1, W2):
    x = np.asarray(x, dtype=np.float32)
    pos = np.asarray(pos, dtype=np.float32)
    ei = np.asarray(edge_index)
    N = x.shape[0]
    dst = ei[:, 0].astype(np.int64)
    src = ei[:, 1].astype(np.int64)

    lay = build_layout(dst, src, N)
    nc = build_program(lay)
    in_maps = _build_inputs(lay, x, pos, np.asarray(expert_weights),
                            np.asarray(gate_W), np.asarray(gate_b),
                            np.asarray(W1), np.asarray(W2))
    res = run_bass_kernel_spmd(nc, in_maps, list(range(N_CORES)))
    out = np.zeros((N, OUT_C), dtype=np.float32)
    for c in range(N_CORES):
        o = res.results[c]["out"]  # [128, CP]
        sel = lay.out_core == c
        nodes = lay.out_node[sel]
        halves = lay.out_half[sel]
        cols = lay.out_col[sel]
        for h in range(2):
            m = halves == h
            out[nodes[m]] = o[h * OUT_C:(h + 1) * OUT_C, cols[m]].T
    return out

